# revision 14
# baseline (speedup 1.0000x reference)
"""DynamicSegmentationHead Trainium2 kernel (bf16 PE path + array tiling).

Data-parallel over the 16 clip-frames: each of the 8 NeuronCores handles 2
frames (100 queries). Per core, the dynamic-conv head is a chain of PE
matmuls over "query groups" (15q -> M=120 rows):

  controller:  params = hs @ W_ctrl.T + b_ctrl   (fp32, host-permuted)
  L1:  y1 = relu(W0eff @ [feat; -gx; -gy; 1])    K=11 (21 mixed), bf16
  L2:  y2 = relu(blockdiag(w1) @ [y1; 1])        K=121, bf16
  L3:  out = blockdiag(w2) @ [y2; 1]             K=121, M=15 col-tiled, bf16

Perf structure vs the fp32r version:
  - all main-loop matmuls in bf16: weight loads overlap with streaming
  - L1 groups sit on 4 distinct 32-row PE row-groups -> up to 4 matmuls
    run concurrently in the array; L3 uses 4 distinct 32-col groups
  - features live in SBUF as bf16 (half the input DMA)
  - PSUM evacuation (relu/copy) split greedily across ACT and DVE;
    output DMAs ride sync+gpsimd queues to keep ACT clean
"""

import numpy as np
import ml_dtypes

import concourse.bass as bass
import concourse.bacc as bacc
import concourse.tile as tile
from concourse import mybir
from concourse import bass_utils

F32 = mybir.dt.float32
BF16 = mybir.dt.bfloat16

HID = 256
NP = 169
Q = 50
H, W = 72, 120
P = H * W            # 8640
NQ = 100             # queries per core (2 frames)
NCORES = 8
STRIDE = 4

CHUNK = 512
CHUNKS = [(i * CHUNK, CHUNK) for i in range(16)] + [(16 * CHUNK, P - 16 * CHUNK)]
MMN = 512            # matmul free-dim window

# group table: (kind, qbase, nq, band) ; kind 0=f0, 1=f1, 2=mixed
# band = partition base of the group's K-rows in Ft / T1 (32-aligned:
# matmul operands must start on a 32-partition boundary).
# Ft bands: 0 = f0, 32 = f1, 64 = mixed [f0;-gx;-gy;1;f1;-gx;-gy], 96 = f1.
GROUPS = [
    (0, 0, 15, 0),      # g0 f0 q0-14
    (0, 15, 15, 64),    # g1 f0 q15-29 (head rows of the mixed band)
    (0, 30, 15, 0),     # g2 f0 q30-44
    (1, 0, 15, 32),     # g3 f1 q0-14
    (1, 15, 15, 96),    # g4 f1 q15-29
    (1, 30, 15, 32),    # g5 f1 q30-44
    (2, 45, 10, 64),    # g6 mixed q45-49 both frames (K=21)
]

ACT_OVH, ACT_RATE = 253.0, 1.0 / 1.2       # measured: ~680ns per [*,512] op
DVE_OVH, DVE_RATE = 148.0, 1.0 / 0.96


def _build_program():
    nc = bacc.Bacc("TRN2", target_bir_lowering=False, debug=False)

    mf = nc.dram_tensor("mf", [2, 8, P], BF16, kind="ExternalInput").ap()
    hsz = nc.dram_tensor("hsz", [HID + 1, NQ], F32, kind="ExternalInput").ap()
    wdev = nc.dram_tensor("wdev", [HID + 1, NP], F32, kind="ExternalInput").ap()
    refs = nc.dram_tensor("refs", [2, 1024], F32, kind="ExternalInput").ap()
    cst = nc.dram_tensor("cst", [3, P], BF16, kind="ExternalInput").ap()
    outp = nc.dram_tensor("outp", [NQ, P], F32, kind="ExternalOutput").ap()

    Relu = mybir.ActivationFunctionType.Relu
    Copy = mybir.ActivationFunctionType.Copy

    # greedy ACT/DVE load balancer (relu + psum-evac copies)
    eng_t = {"act": 0.0, "dve": 6000.0}  # DVE pre-loaded: ctrl copies+memsets

    def pick_engine(fd):
        ca = eng_t["act"] + ACT_OVH + fd * ACT_RATE
        cd = eng_t["dve"] + DVE_OVH + fd * DVE_RATE
        if ca <= cd:
            eng_t["act"] = ca
            return "act"
        eng_t["dve"] = cd
        return "dve"

    def relu_to(out_ap, in_ap, fd):
        if pick_engine(fd) == "act":
            nc.scalar.activation(out_ap, in_ap, Relu)
        else:
            nc.vector.tensor_scalar(
                out=out_ap, in0=in_ap, scalar1=0.0, scalar2=None,
                op0=mybir.AluOpType.max)

    def copy_to(out_ap, in_ap, fd):
        if pick_engine(fd) == "act":
            nc.scalar.activation(out_ap, in_ap, Copy)
        else:
            nc.vector.tensor_copy(out_ap, in_ap)

    with tile.TileContext(nc) as tc:
        with tc.tile_pool(name="persist", bufs=1) as pers, \
             tc.tile_pool(name="stg", bufs=4) as stg:

            # ---------------- persistent SBUF ----------------
            Ft = pers.tile([128, P], BF16, tag="F")
            wsb1 = pers.tile([128, NP], F32, tag="wsb1")
            wsb2 = pers.tile([128, NP], F32, tag="wsb2")
            wsb3 = pers.tile([1, NP], F32, tag="wsb3")
            hsb1 = pers.tile([128, NQ], F32, tag="hsb1")
            hsb2 = pers.tile([128, NQ], F32, tag="hsb2")
            hsb3 = pers.tile([1, NQ], F32, tag="hsb3")
            refsb = pers.tile([2, 1024], F32, tag="refsb")
            PW0S = pers.tile([11, 1024], F32, tag="PW0S")
            PW1S = pers.tile([9, 1024], F32, tag="PW1S")
            PW23S = pers.tile([9, NQ], F32, tag="PW23S")
            ctile = pers.tile([1, 1024], F32, tag="ctile")
            vtmp = pers.tile([3, 1024], F32, tag="vtmp")
            w89 = pers.tile([2, 1024], F32, tag="w89")
            b1raw = pers.tile([1, 1024], F32, tag="b1raw")
            ones3 = pers.tile([3, 1], F32, tag="ones3")
            # bf16 interleaved staging (all partition-base 0)
            Sw = pers.tile([10, NQ * 8], BF16, tag="Sw")     # w0 rows, (n,o)
            Sc = pers.tile([1, NQ * 8], BF16, tag="Sc")      # c row, (n,o)
            W1C2 = pers.tile([8, 840], BF16, tag="W1C2")     # w1 group-ordered
            Sb1 = pers.tile([1, NQ * 8], BF16, tag="Sb1")    # b1 row, (n,o2)
            W23G = pers.tile([9, 105], BF16, tag="W23G")     # w2+b2 grp-ordered

            T1 = pers.tile([128, 840], BF16, tag="T1")
            T2 = pers.tile([121, 840], BF16, tag="T2")
            T3 = pers.tile([121, 112], BF16, tag="T3")
            y1l = [pers.tile([121, MMN], BF16, name=f"y1_{i}", tag=f"y1_{i}")
                   for i in range(8)]
            y2l = [pers.tile([121, MMN], BF16, name=f"y2_{i}", tag=f"y2_{i}")
                   for i in range(8)]
            stageA = pers.tile([128, P], F32, tag="stageA")
            stageB = pers.tile([128, P], F32, tag="stageB")

            # ---------------- input DMAs ----------------
            # controller inputs first (unblock PE asap), on two queues
            nc.sync.dma_start(out=wsb1[:, :], in_=wdev[0:128, :])
            nc.scalar.dma_start(out=wsb2[:, :], in_=wdev[128:256, :])
            nc.sync.dma_start(out=wsb3[:, :], in_=wdev[256:257, :])
            nc.scalar.dma_start(out=hsb1[:, :], in_=hsz[0:128, :])
            nc.sync.dma_start(out=hsb2[:, :], in_=hsz[128:256, :])
            nc.scalar.dma_start(out=hsb3[:, :], in_=hsz[256:257, :])
            nc.sync.dma_start(out=refsb[:, :], in_=refs)

            # Ft band layout (partition: content), all bf16:
            #   0-10   [f0, -gx, -gy, 1]     32-42  [f1, -gx, -gy, 1]
            #   64-84  [f0, -gx, -gy, 1, f1, -gx, -gy]   (mixed, K=21)
            #   96-106 [f1, -gx, -gy, 1]
            fq = [nc.gpsimd, nc.sync, nc.scalar]
            _fi = [0]

            def fdma(**kw):
                fq[_fi[0] % len(fq)].dma_start(**kw)
                _fi[0] += 1

            for base, fr in ((0, 0), (32, 1), (96, 1)):
                fdma(out=Ft[base:base + 8, :], in_=mf[fr])
                fdma(out=Ft[base + 8:base + 10, :], in_=cst[0:2, :])
                fdma(out=Ft[base + 10:base + 11, :], in_=cst[2:3, :])
            fdma(out=Ft[64:72, :], in_=mf[0])
            fdma(out=Ft[72:74, :], in_=cst[0:2, :])
            fdma(out=Ft[74:75, :], in_=cst[2:3, :])
            fdma(out=Ft[75:83, :], in_=mf[1])
            fdma(out=Ft[83:85, :], in_=cst[0:2, :])

            # ---------------- zero fills ----------------
            nc.vector.memset(W23G[:, :], 0.0)
            nc.vector.memset(ones3[:, :], 1.0)
            nc.vector.memset(T2[:, :], 0.0)   # rows 80-119 of g6 cols stay 0
            nc.vector.memset(T3[:, :], 0.0)
            nc.vector.memset(T1[64:85, 720:800], 0.0)  # mixed-group cross rows
            for t in y1l + y2l:
                nc.vector.memset(t[0:120, :], 0.0)
                nc.gpsimd.dma_start(out=t[120:121, :], in_=cst[2:3, 0:MMN])

            # ---------------- controller matmuls (fp32) ----------------
            with tc.tile_pool(name="psctrl", bufs=1, space="PSUM") as psc:
                pw0 = psc.tile([11, 1024], F32, tag="pw0")
                pw1 = psc.tile([9, 1024], F32, tag="pw1")
                pw23 = psc.tile([9, NQ], F32, tag="pw23")
                nc.vector.memset(pw0[:, :], 0.0)
                nc.vector.memset(pw1[:, :], 0.0)
                kchunks = [(wsb1, hsb1, 128), (wsb2, hsb2, 128), (wsb3, hsb3, 1)]
                for o in range(8):
                    for kc, (wk, hk, kn) in enumerate(kchunks):
                        nc.tensor.matmul(
                            pw0[0:11, o * 128:o * 128 + NQ],
                            wk[0:kn, o * 11:o * 11 + 11], hk[0:kn, :],
                            start=(kc == 0), stop=(kc == 2))
                for o2 in range(8):
                    for kc, (wk, hk, kn) in enumerate(kchunks):
                        nc.tensor.matmul(
                            pw1[0:9, o2 * 128:o2 * 128 + NQ],
                            wk[0:kn, 88 + o2 * 9:88 + o2 * 9 + 9], hk[0:kn, :],
                            start=(kc == 0), stop=(kc == 2))
                for kc, (wk, hk, kn) in enumerate(kchunks):
                    nc.tensor.matmul(
                        pw23[0:9, 0:NQ],
                        wk[0:kn, 160:169], hk[0:kn, :],
                        start=(kc == 0), stop=(kc == 2))

                nc.vector.tensor_copy(PW0S[:, :], pw0[:, :])
                nc.vector.tensor_copy(PW1S[:, :], pw1[:, :])
                nc.vector.tensor_copy(PW23S[:, :], pw23[:, :])

                # c row: c[o*128+n] = w8*refx + w9*refy + b0
                nc.sync.dma_start(out=w89[0:2, :], in_=PW0S[8:10, :])
                nc.vector.tensor_tensor(out=vtmp[0:2, :], in0=w89[0:2, :],
                                        in1=refsb[0:2, :],
                                        op=mybir.AluOpType.mult)
                nc.sync.dma_start(out=vtmp[2:3, :], in_=PW0S[10:11, :])
                pc = psc.tile([1, 1024], F32, tag="pc")
                nc.tensor.matmul(pc[0:1, 0:512], ones3[0:3, 0:1],
                                 vtmp[0:3, 0:512], start=True, stop=True)
                nc.tensor.matmul(pc[0:1, 512:1024], ones3[0:3, 0:1],
                                 vtmp[0:3, 512:1024], start=True, stop=True)
                nc.vector.tensor_copy(ctile[0:1, :], pc[0:1, :])

            # ------- interleaved staging (DVE, fp32 -> bf16) -------
            # Sw[r, n*8+o]   = PW0S[r, o*128+n]
            # Sc[0, n*8+o]   = ctile[0, o*128+n]
            # Sb1[0, n*8+o2] = PW1S[8, o2*128+n]
            w0r = PW0S.rearrange("p (o n) -> p n o", o=8)    # [11, 128, 8]
            cr = ctile.rearrange("p (o n) -> p n o", o=8)    # [1, 128, 8]
            w1r = PW1S.rearrange("p (o n) -> p n o", o=8)    # [9, 128, 8]
            nc.vector.tensor_copy(Sw[0:10, :], w0r[0:10, 0:NQ, :])
            nc.vector.tensor_copy(Sc[0:1, :], cr[0:1, 0:NQ, :])
            nc.sync.dma_start(out=b1raw[0:1, :], in_=PW1S[8:9, :])
            b1r = b1raw.rearrange("p (o n) -> p n o", o=8)
            nc.vector.tensor_copy(Sb1[0:1, :], b1r[0:1, 0:NQ, :])
            # group-ordered stagings: cols (g, j, .) ; g6 at block 6
            for dst0, n0, n1 in ((0, 0, 45), (360, 50, 95),
                                 (720, 45, 50), (760, 95, 100)):
                nc.vector.tensor_copy(W1C2[0:8, dst0:dst0 + (n1 - n0) * 8],
                                      w1r[0:8, n0:n1, :])
            for dst0, n0, n1 in ((0, 0, 45), (45, 50, 95),
                                 (90, 45, 50), (95, 95, 100)):
                nc.vector.tensor_copy(W23G[0:9, dst0:dst0 + n1 - n0],
                                      PW23S[0:9, n0:n1])

            # ------------- per-group weight builds (bf16 block DMAs) -----
            _brot = [[nc.sync, nc.scalar], [nc.gpsimd]]
            _bi = [0]
            _bphase = [0]

            def bdma(**kw):
                rot = _brot[_bphase[0]]
                e = rot[_bi[0] % len(rot)]
                _bi[0] += 1
                e.dma_start(**kw)

            # T1: per-group [w0(8); w8; w9; c] at the group's band rows.
            # Sw columns for group g start at (50*kind + qbase)*8.
            # phase-A groups first (g0-g3), then B (g4-g6) on POOL queue.
            def t1_build(g):
                kind, qb, nq, band = GROUPS[g]
                n0 = (50 * (1 if kind == 1 else 0) + qb) * 8
                c0 = g * 120
                m = nq * 8
                if kind < 2:
                    bdma(out=T1[band:band + 10, c0:c0 + m],
                         in_=Sw[0:10, n0:n0 + m])
                    bdma(out=T1[band + 10:band + 11, c0:c0 + m],
                         in_=Sc[0:1, n0:n0 + m])
                else:  # mixed: f0 q45-49 then f1 q45-49, shared c row 74
                    bdma(out=T1[64:74, c0:c0 + 40], in_=Sw[0:10, 360:400])
                    bdma(out=T1[74:75, c0:c0 + 40], in_=Sc[0:1, 360:400])
                    bdma(out=T1[75:85, c0 + 40:c0 + 80], in_=Sw[0:10, 760:800])
                    bdma(out=T1[74:75, c0 + 40:c0 + 80], in_=Sc[0:1, 760:800])

            for g in (0, 1, 2, 3):
                t1_build(g)
            # T2 block-diagonal: one DMA per j covering all groups
            for j in range(15):
                gcnt = 7 if j < 10 else 6
                src = bass.AP(tensor=W1C2.tensor, offset=W1C2.offset + j * 8,
                              ap=[[W1C2.ap[0][0], 8], [120, gcnt], [1, 8]])
                dst = bass.AP(tensor=T2.tensor,
                              offset=T2.offset + j * 8 * T2.ap[0][0] + j * 8,
                              ap=[[T2.ap[0][0], 8], [120, gcnt], [1, 8]])
                bdma(out=dst, in_=src)
            bdma(out=T2[120:121, 0:360], in_=Sb1[0:1, 0:360])
            bdma(out=T2[120:121, 360:720], in_=Sb1[0:1, 400:760])
            # T3 columns: T3[j*8+o2, g*16+j] = w2 ; row 120 = b2
            for j in range(15):
                gcnt = 7 if j < 10 else 6
                srcA = bass.AP(tensor=W23G.tensor, offset=W23G.offset + j,
                               ap=[[W23G.ap[0][0], 8], [15, gcnt], [1, 1]])
                dstA = bass.AP(tensor=T3.tensor,
                               offset=T3.offset + j * 8 * T3.ap[0][0] + j,
                               ap=[[T3.ap[0][0], 8], [16, gcnt], [1, 1]])
                bdma(out=dstA, in_=srcA)
            dstb2 = bass.AP(tensor=T3.tensor, offset=T3.offset + 120 * T3.ap[0][0],
                            ap=[[T3.ap[0][0], 1], [16, 7], [1, 15]])
            srcb2 = bass.AP(tensor=W23G.tensor, offset=W23G.offset + 8 * W23G.ap[0][0],
                            ap=[[W23G.ap[0][0], 1], [15, 7], [1, 15]])
            bdma(out=dstb2, in_=srcb2)
            # ---- phase-B-only builds (POOL SWDGE; hidden under phase A) ----
            _bphase[0] = 1
            for g in (4, 5, 6):
                t1_build(g)
            bdma(out=T2[120:121, 720:760], in_=Sb1[0:1, 360:400])
            bdma(out=T2[120:121, 760:800], in_=Sb1[0:1, 760:800])

            # ---------------- main loop: two phases ----------------
            psm_cm = tc.tile_pool(name="psmain", bufs=3, space="PSUM")
            psm = psm_cm.__enter__()
            QUARTERS = [(0, 4), (4, 8), (8, 12), (12, 17)]  # chunk-index spans
            PHASES = [
                # (glist, stage, strip-dmas: (srow, erow, orow))
                ([0, 1, 3, 4], stageA,
                 [(0, 15, 0), (32, 47, 15), (64, 79, 50), (96, 111, 65)]),
                ([2, 5, 6], stageB,
                 [(0, 15, 30), (32, 47, 80), (64, 69, 45), (69, 74, 95)]),
            ]
            out_eng = [nc.sync, nc.gpsimd]
            for pi, (glist, stage_t, strips) in enumerate(PHASES):
                glen = len(glist)
                for ci, (coff, clen) in enumerate(CHUNKS):
                    ps1s = {}
                    # L1: all groups back-to-back (distinct row-groups ->
                    # the PE runs them concurrently)
                    for gi, g in enumerate(glist):
                        kind, qb, nq, band = GROUPS[g]
                        m = nq * 8
                        k1 = 21 if kind == 2 else 11
                        ps1 = psm.tile([128, MMN], F32, tag="ps1", bufs=4,
                                       name=f"ps1_{pi}_{ci}_{gi}")
                        nc.tensor.matmul(
                            ps1[0:m, 0:clen],
                            T1[band:band + k1, g * 120:g * 120 + m],
                            Ft[band:band + k1, coff:coff + clen],
                            start=True, stop=True,
                            tile_position=(band - band % 32, 0))
                        ps1s[g] = ps1
                    # relu1 evacuations
                    y1s = {}
                    for gi, g in enumerate(glist):
                        m = GROUPS[g][2] * 8
                        y1 = y1l[(ci * glen + gi) % 8]
                        relu_to(y1[0:m, 0:clen], ps1s[g][0:m, 0:clen], clen)
                        y1s[g] = y1
                    # L2 (full-K matmuls, serial on the array)
                    ps2s = {}
                    for gi, g in enumerate(glist):
                        m = GROUPS[g][2] * 8
                        ps2 = psm.tile([128, MMN], F32, tag="ps2", bufs=2,
                                       name=f"ps2_{pi}_{ci}_{gi}")
                        nc.tensor.matmul(
                            ps2[0:m, 0:clen],
                            T2[0:121, g * 120:g * 120 + m],
                            y1s[g][0:121, 0:clen],
                            start=True, stop=True)
                        ps2s[g] = ps2
                    # relu2 evacuations
                    y2s = {}
                    for gi, g in enumerate(glist):
                        m = GROUPS[g][2] * 8
                        y2 = y2l[(ci * glen + gi) % 8]
                        relu_to(y2[0:m, 0:clen], ps2s[g][0:m, 0:clen], clen)
                        y2s[g] = y2
                    # L3: col-tiled concurrent matmuls into strip slots
                    mtop = 32 * (glen - 1) + GROUPS[glist[-1]][2]
                    ph = psm.tile([128, MMN], F32, tag="ps3", bufs=2,
                                  name=f"ps3_{pi}_{ci}")
                    for gi, g in enumerate(glist):
                        nq = GROUPS[g][2]
                        s = 32 * gi
                        nc.tensor.matmul(
                            ph[s:s + nq, 0:clen],
                            T3[0:121, g * 16:g * 16 + nq],
                            y2s[g][0:121, 0:clen],
                            start=True, stop=True,
                            tile_position=(0, s),
                            skip_group_check=True)
                    copy_to(stage_t[0:mtop, coff:coff + clen],
                            ph[0:mtop, 0:clen], clen)

                    # quarter boundary: flush this phase's finished columns
                    for qi, (c0i, c1i) in enumerate(QUARTERS):
                        if ci == c1i - 1:
                            p0 = CHUNKS[c0i][0]
                            p1 = coff + clen
                            for si, (srow, erow, orow) in enumerate(strips):
                                nrow = erow - srow
                                out_eng[si % 2].dma_start(
                                    out=outp[orow:orow + nrow, p0:p1],
                                    in_=stage_t[srow:erow, p0:p1])
            psm_cm.__exit__(None, None, None)

    nc.compile()
    return nc


_NC = None


def _get_nc():
    global _NC
    if _NC is None:
        _NC = _build_program()
    return _NC


def _host_pack(hs, mask_features, references, sizes, W_ctrl, b_ctrl):
    hs = np.asarray(hs, np.float32)
    mask_features = np.asarray(mask_features, np.float32)
    references = np.asarray(references, np.float32)
    sizes = np.asarray(sizes, np.float32)
    W_ctrl = np.asarray(W_ctrl, np.float32)
    b_ctrl = np.asarray(b_ctrl, np.float32)

    # pixel grid (bf16 rows: -gx, -gy, ones)
    xs = np.arange(W, dtype=np.float32) * STRIDE + STRIDE // 2
    ys = np.arange(H, dtype=np.float32) * STRIDE + STRIDE // 2
    gxf = np.tile(xs, H)
    gyf = np.repeat(ys, W)
    cstm = np.stack([-gxf, -gyf, np.ones(P, np.float32)]).astype(ml_dtypes.bfloat16)

    # W permutation
    W_aug = np.concatenate([W_ctrl.T, b_ctrl[None, :]], 0)  # [257, 169]
    perm = []
    for o in range(8):
        perm += [o * 10 + i for i in range(10)] + [152 + o]
    for o2 in range(8):
        perm += [80 + o2 * 8 + oo for oo in range(8)] + [160 + o2]
    perm += [144 + oo for oo in range(8)] + [168]
    wdev = np.ascontiguousarray(W_aug[:, perm])

    # reference points in pixels
    b_idx = np.arange(16) // 8
    scale = sizes[b_idx][:, ::-1]                      # [16, 2] = (img_w, img_h)
    refs_px = references * scale[:, None, :]           # [16, 50, 2]

    in_maps = []
    for c in range(NCORES):
        hs_c = hs[2 * c:2 * c + 2].reshape(NQ, HID)
        hsz = np.concatenate([hs_c.T, np.ones((1, NQ), np.float32)], 0)
        mf_c = mask_features[2 * c:2 * c + 2].reshape(2, 8, P).astype(
            ml_dtypes.bfloat16)
        rp = refs_px[2 * c:2 * c + 2].reshape(NQ, 2)
        refs_rep = np.zeros((2, 1024), np.float32)
        for o in range(8):
            refs_rep[0, o * 128:o * 128 + NQ] = rp[:, 0]
            refs_rep[1, o * 128:o * 128 + NQ] = rp[:, 1]
        in_maps.append(dict(
            mf=np.ascontiguousarray(mf_c),
            hsz=np.ascontiguousarray(hsz),
            wdev=wdev,
            refs=refs_rep,
            cst=cstm,
        ))
    return in_maps


def kernel(hs, mask_features, references, sizes, W_ctrl, b_ctrl, T):
    assert int(T) == 8
    nc = _get_nc()
    in_maps = _host_pack(hs, mask_features, references, sizes, W_ctrl, b_ctrl)
    res = bass_utils.run_bass_kernel_spmd(nc, in_maps, core_ids=list(range(NCORES)))
    out = np.empty((16, Q, H, W), np.float32)
    for c in range(NCORES):
        out[2 * c:2 * c + 2] = res.results[c]["outp"].reshape(2, Q, H, W)
    return out


# revision 16
# speedup vs baseline: 1.3746x; 1.3746x over previous
"""DynamicSegmentationHead Trainium2 kernel (fp16 PE path, wide-evac).

Data-parallel over the 16 clip-frames: each of the 8 NeuronCores handles 2
frames (100 queries). Per core, the head is a chain of PE matmuls over
"query groups" (15q -> M=120 rows):

  controller:  params = hs @ W_ctrl.T + b_ctrl   (fp16 in, fp32 psum)
  L1:  y1 = relu(W0eff @ [feat; -gx; -gy; 1])    K=11 (21 mixed)
  L2:  y2 = relu(blockdiag(w1|b1) @ [y1; 1])     K=121 (bias via ones row)
  L3:  out = blockdiag(w2|b2) @ [y2; 1]          K=121, M=15 col-tiled

Perf structure:
  - whole matmul path in fp16 (1 cycle/row, rel-err ~1e-3)
  - the dense controller matmul stream doubles as the HAM warmup; dummy
    matmuls bridge the staging gap so the PE array stays at full clock
  - L1 groups sit on distinct 32-row PE row-groups, L3 on distinct
    32-col groups -> concurrent execution in the array
  - PSUM pair tiles [128,1024]: two groups share one tile, evacuated by
    a single wide ACT/DVE op (relu+copy are the true bottleneck)
  - output DMAs ride sync+gpsimd queues, keeping ACT/DVE clean
"""

import numpy as np

import concourse.bass as bass
import concourse.bacc as bacc
import concourse.tile as tile
from concourse import mybir
from concourse import bass_utils

F32 = mybir.dt.float32
F16 = mybir.dt.float16

HID = 256
NP = 169
Q = 50
H, W = 72, 120
P = H * W            # 8640
NQ = 100             # queries per core (2 frames)
NCORES = 8
STRIDE = 4

CHUNK = 512
CHUNKS = [(i * CHUNK, CHUNK) for i in range(16)] + [(16 * CHUNK, P - 16 * CHUNK)]
MMN = 512

# group table: (kind, qbase, nq, band) ; kind 0=f0, 1=f1, 2=mixed
# band = 32-aligned partition base of the group's K-rows in Ft / T1.
# Ft bands: 0 = f0, 32 = f1, 64 = mixed [f0;-gx;-gy;1;f1;-gx;-gy], 96 = f1.
GROUPS = [
    (0, 0, 15, 0),      # g0 f0 q0-14
    (0, 15, 15, 64),    # g1 f0 q15-29 (head rows of the mixed band)
    (0, 30, 15, 0),     # g2 f0 q30-44
    (1, 0, 15, 32),     # g3 f1 q0-14
    (1, 15, 15, 96),    # g4 f1 q15-29
    (1, 30, 15, 32),    # g5 f1 q30-44
    (2, 45, 10, 64),    # g6 mixed q45-49 both frames (K=21)
]

ACT_OVH, ACT_RATE = 352.0 / 1.2, 1.0 / 1.2
DVE_OVH, DVE_RATE = 150.0, 1.0 / 0.96


def _build_program():
    nc = bacc.Bacc("TRN2", target_bir_lowering=False, debug=False)

    mf = nc.dram_tensor("mf", [2, 8, P], F16, kind="ExternalInput").ap()
    hsz = nc.dram_tensor("hsz", [HID + 1, NQ], F16, kind="ExternalInput").ap()
    wdev = nc.dram_tensor("wdev", [HID + 1, NP], F16, kind="ExternalInput").ap()
    refs = nc.dram_tensor("refs", [2, 1024], F32, kind="ExternalInput").ap()
    cst = nc.dram_tensor("cst", [3, P], F16, kind="ExternalInput").ap()
    outp = nc.dram_tensor("outp", [NQ, P], F32, kind="ExternalOutput").ap()

    Relu = mybir.ActivationFunctionType.Relu
    Copy = mybir.ActivationFunctionType.Copy

    eng_t = {"act": 0.0, "dve": 6000.0}  # DVE preloaded: ctrl copies+memsets

    def pick_engine(fd):
        ca = eng_t["act"] + ACT_OVH + fd * ACT_RATE
        cd = eng_t["dve"] + DVE_OVH + fd * DVE_RATE
        if ca <= cd:
            eng_t["act"] = ca
            return "act"
        eng_t["dve"] = cd
        return "dve"

    def relu_to(out_ap, in_ap, fd):
        if pick_engine(fd) == "act":
            nc.scalar.activation(out_ap, in_ap, Relu)
        else:
            nc.vector.tensor_scalar(
                out=out_ap, in0=in_ap, scalar1=0.0, scalar2=None,
                op0=mybir.AluOpType.max)

    def copy_to(out_ap, in_ap, fd):
        if pick_engine(fd) == "act":
            nc.scalar.activation(out_ap, in_ap, Copy)
        else:
            nc.vector.tensor_copy(out_ap, in_ap)

    with tile.TileContext(nc) as tc:
        with tc.tile_pool(name="persist", bufs=1) as pers:

            # ---------------- persistent SBUF ----------------
            Ft = pers.tile([128, P], F16, tag="F")
            wsb1 = pers.tile([128, NP], F16, tag="wsb1")
            wsb2 = pers.tile([128, NP], F16, tag="wsb2")
            wsb3 = pers.tile([1, NP], F16, tag="wsb3")
            hsb1 = pers.tile([128, NQ], F16, tag="hsb1")
            hsb2 = pers.tile([128, NQ], F16, tag="hsb2")
            hsb3 = pers.tile([1, NQ], F16, tag="hsb3")
            refsb = pers.tile([2, 1024], F32, tag="refsb")
            PW0S = pers.tile([11, 1024], F32, tag="PW0S")
            PW1S = pers.tile([9, 1024], F32, tag="PW1S")
            PW23S = pers.tile([9, NQ], F32, tag="PW23S")
            ctile = pers.tile([1, 1024], F32, tag="ctile")
            vtmp = pers.tile([3, 1024], F32, tag="vtmp")
            w89 = pers.tile([2, 1024], F32, tag="w89")
            b1raw = pers.tile([1, 1024], F32, tag="b1raw")
            ones3 = pers.tile([3, 1], F32, tag="ones3")
            # fp16 interleaved staging (all partition-base 0)
            Sw = pers.tile([10, NQ * 8], F16, tag="Sw")      # w0 rows, (n,o)
            Sc = pers.tile([1, NQ * 8], F16, tag="Sc")       # c row, (n,o)
            W1C2 = pers.tile([8, 840], F16, tag="W1C2")      # w1 group-ordered
            Sb1 = pers.tile([1, NQ * 8], F16, tag="Sb1")     # b1 row, (n,o2)
            W23G = pers.tile([9, 105], F16, tag="W23G")      # w2+b2 grp-ordered

            T1 = pers.tile([128, 840], F16, tag="T1")
            T2 = pers.tile([121, 840], F16, tag="T2")
            T3 = pers.tile([121, 112], F16, tag="T3")
            y1l = [pers.tile([121, 1024], F16, name=f"y1_{i}", tag=f"y1_{i}")
                   for i in range(4)]
            y2l = [pers.tile([121, 1024], F16, name=f"y2_{i}", tag=f"y2_{i}")
                   for i in range(4)]
            stageA = pers.tile([128, P], F32, tag="stageA")
            stageB = pers.tile([128, P], F32, tag="stageB")

            # ---------------- input DMAs ----------------
            # controller inputs first (unblock PE asap), on two queues
            nc.sync.dma_start(out=wsb1[:, :], in_=wdev[0:128, :])
            nc.scalar.dma_start(out=wsb2[:, :], in_=wdev[128:256, :])
            nc.sync.dma_start(out=wsb3[:, :], in_=wdev[256:257, :])
            nc.scalar.dma_start(out=hsb1[:, :], in_=hsz[0:128, :])
            nc.sync.dma_start(out=hsb2[:, :], in_=hsz[128:256, :])
            nc.scalar.dma_start(out=hsb3[:, :], in_=hsz[256:257, :])
            nc.sync.dma_start(out=refsb[:, :], in_=refs)

            fq = [nc.gpsimd, nc.sync, nc.scalar]
            _fi = [0]

            def fdma(**kw):
                fq[_fi[0] % len(fq)].dma_start(**kw)
                _fi[0] += 1

            for base, fr in ((0, 0), (32, 1), (96, 1)):
                fdma(out=Ft[base:base + 8, :], in_=mf[fr])
                fdma(out=Ft[base + 8:base + 10, :], in_=cst[0:2, :])
                fdma(out=Ft[base + 10:base + 11, :], in_=cst[2:3, :])
            fdma(out=Ft[64:72, :], in_=mf[0])
            fdma(out=Ft[72:74, :], in_=cst[0:2, :])
            fdma(out=Ft[74:75, :], in_=cst[2:3, :])
            fdma(out=Ft[75:83, :], in_=mf[1])
            fdma(out=Ft[83:85, :], in_=cst[0:2, :])

            # ---------------- zero fills ----------------
            nc.vector.memset(W23G[:, :], 0.0)
            nc.vector.memset(ones3[:, :], 1.0)
            nc.vector.memset(T2[:, :], 0.0)   # rows 80-119 of g6 cols stay 0
            nc.vector.memset(T3[:, :], 0.0)
            nc.vector.memset(T1[64:85, 720:800], 0.0)  # mixed-group cross rows
            for t in y1l + y2l:
                nc.vector.memset(t[0:120, :], 0.0)
                nc.gpsimd.dma_start(out=t[120:121, :], in_=cst[2:3, 0:1024])

            # ------------- controller matmuls (fp16, HAM warmup) ---------
            with tc.tile_pool(name="psctrl", bufs=1, space="PSUM") as psc:
                pw0 = psc.tile([11, 1024], F32, tag="pw0")
                pw1 = psc.tile([9, 1024], F32, tag="pw1")
                pw23 = psc.tile([9, NQ], F32, tag="pw23")
                nc.vector.memset(pw0[:, :], 0.0)
                nc.vector.memset(pw1[:, :], 0.0)
                kchunks = [(wsb1, hsb1, 128), (wsb2, hsb2, 128), (wsb3, hsb3, 1)]
                for o in range(8):
                    for kc, (wk, hk, kn) in enumerate(kchunks):
                        nc.tensor.matmul(
                            pw0[0:11, o * 128:o * 128 + NQ],
                            wk[0:kn, o * 11:o * 11 + 11], hk[0:kn, :],
                            start=(kc == 0), stop=(kc == 2))
                for o2 in range(8):
                    for kc, (wk, hk, kn) in enumerate(kchunks):
                        nc.tensor.matmul(
                            pw1[0:9, o2 * 128:o2 * 128 + NQ],
                            wk[0:kn, 88 + o2 * 9:88 + o2 * 9 + 9], hk[0:kn, :],
                            start=(kc == 0), stop=(kc == 2))
                for kc, (wk, hk, kn) in enumerate(kchunks):
                    nc.tensor.matmul(
                        pw23[0:9, 0:NQ],
                        wk[0:kn, 160:169], hk[0:kn, :],
                        start=(kc == 0), stop=(kc == 2))

                nc.vector.tensor_copy(PW0S[:, :], pw0[:, :])
                nc.scalar.activation(PW1S[:, :], pw1[:, :], Copy)
                nc.scalar.activation(PW23S[:, :], pw23[:, :], Copy)

                # c row: c[o*128+n] = w8*refx + w9*refy + b0
                nc.sync.dma_start(out=w89[0:2, :], in_=PW0S[8:10, :])
                nc.vector.tensor_tensor(out=vtmp[0:2, :], in0=w89[0:2, :],
                                        in1=refsb[0:2, :],
                                        op=mybir.AluOpType.mult)
                nc.sync.dma_start(out=vtmp[2:3, :], in_=PW0S[10:11, :])
                pc = psc.tile([1, 1024], F32, tag="pc")
                nc.tensor.matmul(pc[0:1, 0:512], ones3[0:3, 0:1],
                                 vtmp[0:3, 0:512], start=True, stop=True)
                nc.tensor.matmul(pc[0:1, 512:1024], ones3[0:3, 0:1],
                                 vtmp[0:3, 512:1024], start=True, stop=True)
                nc.vector.tensor_copy(ctile[0:1, :], pc[0:1, :])

                # dummy matmuls: keep the PE array busy (HAM at full clock)
                # while DVE/DMA stage the per-group weights below.
                pwm = psc.tile([128, MMN], F32, tag="pwm")
                for wi in range(14):
                    nc.tensor.matmul(
                        pwm[0:100, 0:MMN],
                        hsb1[0:128, 0:NQ], Ft[0:128, 0:MMN],
                        start=True, stop=True)

            # ------- interleaved staging (DVE, fp32 -> fp16) -------
            w0r = PW0S.rearrange("p (o n) -> p n o", o=8)    # [11, 128, 8]
            cr = ctile.rearrange("p (o n) -> p n o", o=8)    # [1, 128, 8]
            w1r = PW1S.rearrange("p (o n) -> p n o", o=8)    # [9, 128, 8]
            nc.vector.tensor_copy(Sw[0:10, :], w0r[0:10, 0:NQ, :])
            nc.vector.tensor_copy(Sc[0:1, :], cr[0:1, 0:NQ, :])
            nc.sync.dma_start(out=b1raw[0:1, :], in_=PW1S[8:9, :])
            b1r = b1raw.rearrange("p (o n) -> p n o", o=8)
            nc.vector.tensor_copy(Sb1[0:1, :], b1r[0:1, 0:NQ, :])
            # group-ordered stagings: cols (g, j, .) ; g6 at block 6
            for dst0, n0, n1 in ((0, 0, 45), (360, 50, 95),
                                 (720, 45, 50), (760, 95, 100)):
                nc.vector.tensor_copy(W1C2[0:8, dst0:dst0 + (n1 - n0) * 8],
                                      w1r[0:8, n0:n1, :])
            for dst0, n0, n1 in ((0, 0, 45), (45, 50, 95),
                                 (90, 45, 50), (95, 95, 100)):
                nc.scalar.activation(W23G[0:9, dst0:dst0 + n1 - n0],
                                     PW23S[0:9, n0:n1], Copy)

            # ------------- per-group weight builds (fp16 block DMAs) -----
            _brot = [[nc.sync, nc.scalar], [nc.gpsimd]]
            _bi = [0]
            _bphase = [0]

            def bdma(**kw):
                rot = _brot[_bphase[0]]
                e = rot[_bi[0] % len(rot)]
                _bi[0] += 1
                e.dma_start(**kw)

            def t1_build(g):
                kind, qb, nq, band = GROUPS[g]
                n0 = (50 * (1 if kind == 1 else 0) + qb) * 8
                c0 = g * 120
                m = nq * 8
                if kind < 2:
                    bdma(out=T1[band:band + 10, c0:c0 + m],
                         in_=Sw[0:10, n0:n0 + m])
                    bdma(out=T1[band + 10:band + 11, c0:c0 + m],
                         in_=Sc[0:1, n0:n0 + m])
                else:  # mixed: f0 q45-49 then f1 q45-49, shared c row 74
                    bdma(out=T1[64:74, c0:c0 + 40], in_=Sw[0:10, 360:400])
                    bdma(out=T1[74:75, c0:c0 + 40], in_=Sc[0:1, 360:400])
                    bdma(out=T1[75:85, c0 + 40:c0 + 80], in_=Sw[0:10, 760:800])
                    bdma(out=T1[74:75, c0 + 40:c0 + 80], in_=Sc[0:1, 760:800])

            for g in (0, 1, 3, 4):   # phase-A groups first
                t1_build(g)
            for j in range(15):
                gcnt = 7 if j < 10 else 6
                src = bass.AP(tensor=W1C2.tensor, offset=W1C2.offset + j * 8,
                              ap=[[W1C2.ap[0][0], 8], [120, gcnt], [1, 8]])
                dst = bass.AP(tensor=T2.tensor,
                              offset=T2.offset + j * 8 * T2.ap[0][0] + j * 8,
                              ap=[[T2.ap[0][0], 8], [120, gcnt], [1, 8]])
                bdma(out=dst, in_=src)
            bdma(out=T2[120:121, 0:360], in_=Sb1[0:1, 0:360])
            bdma(out=T2[120:121, 360:720], in_=Sb1[0:1, 400:760])
            for j in range(15):
                gcnt = 7 if j < 10 else 6
                srcA = bass.AP(tensor=W23G.tensor, offset=W23G.offset + j,
                               ap=[[W23G.ap[0][0], 8], [15, gcnt], [1, 1]])
                dstA = bass.AP(tensor=T3.tensor,
                               offset=T3.offset + j * 8 * T3.ap[0][0] + j,
                               ap=[[T3.ap[0][0], 8], [16, gcnt], [1, 1]])
                bdma(out=dstA, in_=srcA)
            dstb2 = bass.AP(tensor=T3.tensor, offset=T3.offset + 120 * T3.ap[0][0],
                            ap=[[T3.ap[0][0], 1], [16, 7], [1, 15]])
            srcb2 = bass.AP(tensor=W23G.tensor, offset=W23G.offset + 8 * W23G.ap[0][0],
                            ap=[[W23G.ap[0][0], 1], [15, 7], [1, 15]])
            bdma(out=dstb2, in_=srcb2)
            # ---- phase-B-only builds (POOL SWDGE; hidden under phase A) ----
            _bphase[0] = 1
            for g in (2, 5, 6):
                t1_build(g)
            bdma(out=T2[120:121, 720:760], in_=Sb1[0:1, 360:400])
            bdma(out=T2[120:121, 760:800], in_=Sb1[0:1, 760:800])

            # ---------------- main loop: two phases ----------------
            # Pair tiles: two groups share one [128,1024] psum tile (2 banks)
            # and one [121,1024] fp16 y tile; a single wide ACT/DVE op
            # evacuates both groups at once.
            psm_cm = tc.tile_pool(name="psmain", bufs=2, space="PSUM")
            psm = psm_cm.__enter__()
            QUARTERS = [(0, 4), (4, 8), (8, 12), (12, 17)]
            PHASES = [
                # (pairs, stage, strips) ; pair = tuple of groups
                ([(0, 1), (3, 4)], stageA,
                 [(0, 15, 0), (32, 47, 15), (64, 79, 50), (96, 111, 65)]),
                ([(2, 5), (6,)], stageB,
                 [(0, 15, 30), (32, 47, 80), (64, 69, 45), (69, 74, 95)]),
            ]
            out_eng = [nc.sync, nc.gpsimd]
            for pi, (pairs, stage_t, strips) in enumerate(PHASES):
                glist = [g for pr in pairs for g in pr]
                glen = len(glist)
                for ci, (coff, clen) in enumerate(CHUNKS):
                    # L1: all groups back-to-back; distinct row-groups run
                    # concurrently in the array
                    ps1s = []
                    for wi, pr in enumerate(pairs):
                        ps1 = psm.tile([128, 1024], F32, tag="ps1", bufs=2,
                                       name=f"ps1_{pi}_{ci}_{wi}")
                        for w, g in enumerate(pr):
                            kind, qb, nq, band = GROUPS[g]
                            m = nq * 8
                            k1 = 21 if kind == 2 else 11
                            nc.tensor.matmul(
                                ps1[0:m, w * 512:w * 512 + clen],
                                T1[band:band + k1, g * 120:g * 120 + m],
                                Ft[band:band + k1, coff:coff + clen],
                                start=True, stop=True,
                                tile_position=(band, 0))
                        ps1s.append(ps1)
                    # wide relu1 evacuations (one op per pair)
                    y1s = []
                    for wi, pr in enumerate(pairs):
                        y1 = y1l[(ci * 2 + wi) % 4]
                        fd = 512 + clen if len(pr) == 2 else clen
                        mrow = 120 if len(pr) == 2 else GROUPS[pr[0]][2] * 8
                        relu_to(y1[0:mrow, 0:fd], ps1s[wi][0:mrow, 0:fd], fd)
                        y1s.append(y1)
                    # L2 (full-K serial matmuls, bias via ones row)
                    ps2s = []
                    for wi, pr in enumerate(pairs):
                        ps2 = psm.tile([128, 1024], F32, tag="ps2", bufs=2,
                                       name=f"ps2_{pi}_{ci}_{wi}")
                        for w, g in enumerate(pr):
                            m = GROUPS[g][2] * 8
                            nc.tensor.matmul(
                                ps2[0:m, w * 512:w * 512 + clen],
                                T2[0:121, g * 120:g * 120 + m],
                                y1s[wi][0:121, w * 512:w * 512 + clen],
                                start=True, stop=True)
                        ps2s.append(ps2)
                    # wide relu2 evacuations
                    y2s = []
                    for wi, pr in enumerate(pairs):
                        y2 = y2l[(ci * 2 + wi) % 4]
                        fd = 512 + clen if len(pr) == 2 else clen
                        mrow = 120 if len(pr) == 2 else GROUPS[pr[0]][2] * 8
                        relu_to(y2[0:mrow, 0:fd], ps2s[wi][0:mrow, 0:fd], fd)
                        y2s.append(y2)
                    # L3: col-tiled concurrent matmuls into strip slots
                    mtop = 32 * (glen - 1) + GROUPS[glist[-1]][2]
                    ph = psm.tile([128, 1024], F32, tag="ps2", bufs=2,
                                  name=f"ps3_{pi}_{ci}")
                    gi = 0
                    for wi, pr in enumerate(pairs):
                        for w, g in enumerate(pr):
                            nqg = GROUPS[g][2]
                            s = 32 * gi
                            gi += 1
                            nc.tensor.matmul(
                                ph[s:s + nqg, 0:clen],
                                T3[0:121, g * 16:g * 16 + nqg],
                                y2s[wi][0:121, w * 512:w * 512 + clen],
                                start=True, stop=True,
                                tile_position=(0, s),
                                skip_group_check=True)
                    copy_to(stage_t[0:mtop, coff:coff + clen],
                            ph[0:mtop, 0:clen], clen)

                    # quarter boundary: flush this phase's finished columns
                    for qi, (c0i, c1i) in enumerate(QUARTERS):
                        if ci == c1i - 1:
                            p0 = CHUNKS[c0i][0]
                            p1 = coff + clen
                            for si, (srow, erow, orow) in enumerate(strips):
                                nrow = erow - srow
                                out_eng[si % 2].dma_start(
                                    out=outp[orow:orow + nrow, p0:p1],
                                    in_=stage_t[srow:erow, p0:p1])
            psm_cm.__exit__(None, None, None)

    nc.compile()
    return nc


_NC = None


def _get_nc():
    global _NC
    if _NC is None:
        _NC = _build_program()
    return _NC


def _host_pack(hs, mask_features, references, sizes, W_ctrl, b_ctrl):
    hs = np.asarray(hs, np.float32)
    mask_features = np.asarray(mask_features, np.float32)
    references = np.asarray(references, np.float32)
    sizes = np.asarray(sizes, np.float32)
    W_ctrl = np.asarray(W_ctrl, np.float32)
    b_ctrl = np.asarray(b_ctrl, np.float32)

    # pixel grid (fp16 rows: -gx, -gy, ones)
    xs = np.arange(W, dtype=np.float32) * STRIDE + STRIDE // 2
    ys = np.arange(H, dtype=np.float32) * STRIDE + STRIDE // 2
    gxf = np.tile(xs, H)
    gyf = np.repeat(ys, W)
    cstm = np.stack([-gxf, -gyf, np.ones(P, np.float32)]).astype(np.float16)

    # W permutation
    W_aug = np.concatenate([W_ctrl.T, b_ctrl[None, :]], 0)  # [257, 169]
    perm = []
    for o in range(8):
        perm += [o * 10 + i for i in range(10)] + [152 + o]
    for o2 in range(8):
        perm += [80 + o2 * 8 + oo for oo in range(8)] + [160 + o2]
    perm += [144 + oo for oo in range(8)] + [168]
    wdev = np.ascontiguousarray(W_aug[:, perm]).astype(np.float16)

    # reference points in pixels
    b_idx = np.arange(16) // 8
    scale = sizes[b_idx][:, ::-1]                      # [16, 2] = (img_w, img_h)
    refs_px = references * scale[:, None, :]           # [16, 50, 2]

    in_maps = []
    for c in range(NCORES):
        hs_c = hs[2 * c:2 * c + 2].reshape(NQ, HID)
        hsz = np.concatenate([hs_c.T, np.ones((1, NQ), np.float32)],
                             0).astype(np.float16)
        mf_c = mask_features[2 * c:2 * c + 2].reshape(2, 8, P).astype(np.float16)
        rp = refs_px[2 * c:2 * c + 2].reshape(NQ, 2)
        refs_rep = np.zeros((2, 1024), np.float32)
        for o in range(8):
            refs_rep[0, o * 128:o * 128 + NQ] = rp[:, 0]
            refs_rep[1, o * 128:o * 128 + NQ] = rp[:, 1]
        in_maps.append(dict(
            mf=np.ascontiguousarray(mf_c),
            hsz=np.ascontiguousarray(hsz),
            wdev=wdev,
            refs=refs_rep,
            cst=cstm,
        ))
    return in_maps


def kernel(hs, mask_features, references, sizes, W_ctrl, b_ctrl, T):
    assert int(T) == 8
    nc = _get_nc()
    in_maps = _host_pack(hs, mask_features, references, sizes, W_ctrl, b_ctrl)
    res = bass_utils.run_bass_kernel_spmd(nc, in_maps, core_ids=list(range(NCORES)))
    out = np.empty((16, Q, H, W), np.float32)
    for c in range(NCORES):
        out[2 * c:2 * c + 2] = res.results[c]["outp"].reshape(2, Q, H, W)
    return out
